# revision 29
# baseline (speedup 1.0000x reference)
"""Multi-head attention (B=2, S=2048, E=1024, H=16) on 8 TRN2 NeuronCores.

Sharding: batch x head-group. Core c handles batch c//4 and heads
(c%4)*4 .. +3. Each core computes Q/K/V projections, masked softmax
attention, and a partial output projection against its 4-head slice of
Wo (rows h0*64 .. h0*64+255). Host sums the 4 partials per batch and
adds the constant row bo + tile(bv,H) @ Wo (the V-bias contribution
commutes through softmax normalization: PV(v0 + 1*bv) =
PV(v0) + denom*bv, so after normalization it is exactly +bv per head,
which flows through Wo as a constant row).

Device layout is fully transposed (feature-major): the host supplies
x^T and a pre-arranged mask^T so no on-device transposes are needed.
  - qT/kT [KD, S] per head (bf16), computed as W.T @ x^T (f32r matmul;
    both pairs of a group read xt at base partitions 0/64, with the
    weight rows duplicated [W;W] so lhsT/rhs base partitions match)
  - simT [k, q] tiles in PSUM; mask-multiplied on DVE (psum x bf16 ->
    bf16); exp on ACT in chunks
  - PV with stationary [s_chunk, 65] = [V | ones] bf16: row 64 of the
    psum accumulator is the softmax denominator for free
  - reciprocal via a [1,N] -> [128,N/128] DRAM-bounce repack (DVE
    reciprocal is per-lane; a row on 1 partition would be 8 cyc/elem)
  - rec broadcast to 64 partitions via a partition-step-0 DRAM read
  - per-pair outT rows packed into a [128, S] group tile via
    partition-shift SBUF->SBUF DMA; Wo contraction then runs K=128
    (two heads per matmul) with bf16 operands; partials exported bf16
"""
import sys

if "/opt/trn_rl_repo" not in sys.path:
    sys.path.insert(0, "/opt/trn_rl_repo")

from contextlib import ExitStack

import ml_dtypes
import numpy as np

B, S, E = 2, 2048, 1024
H = 16
HD = 64
KD = 64
VD = 64
SCALE = 1.0 / np.float32(np.sqrt(np.float32(KD)))
N_CORES = 8
HPC = H // 4  # heads per core = 4
QB = 512  # q-block width
NKC = S // 128  # 16 k-chunks
NEXP = 4  # exp ops per (pair, qb) pt tile

_RUNTIME = {}


def _build_nc(repeat=1):
    import concourse.bass as bass
    import concourse.tile as tile
    from concourse import mybir, bacc

    F32 = mybir.dt.float32
    F32R = mybir.dt.float32r
    BF16 = mybir.dt.bfloat16
    Copy = mybir.ActivationFunctionType.Copy
    Ident = mybir.ActivationFunctionType.Identity
    Exp = mybir.ActivationFunctionType.Exp

    nc = bacc.Bacc("TRN2")
    xt_d = nc.dram_tensor("xt", (4 * HD, S), F32, kind="ExternalInput")
    mask_d = nc.dram_tensor("maskt", (128, NKC * S), BF16, kind="ExternalInput")
    wp_d = nc.dram_tensor("wpack", (128, 3 * 64 + 2), F32, kind="ExternalInput")
    wo_d = nc.dram_tensor("wo", (4 * VD, E), BF16, kind="ExternalInput")
    out_d = nc.dram_tensor("partial", (S, E), BF16, kind="ExternalOutput")

    with tile.TileContext(nc) as tc:
        with ExitStack() as ctx:
            const = ctx.enter_context(tc.tile_pool(name="const", bufs=1))
            qkp = ctx.enter_context(tc.tile_pool(name="qkp", bufs=2))
            vp = ctx.enter_context(tc.tile_pool(name="vp", bufs=3))
            ptp = ctx.enter_context(tc.tile_pool(name="ptp", bufs=2))
            otp = ctx.enter_context(tc.tile_pool(name="otp", bufs=4))
            ot2p = ctx.enter_context(tc.tile_pool(name="ot2p", bufs=2))
            denp_pool = ctx.enter_context(tc.tile_pool(name="denp", bufs=3))
            wst = ctx.enter_context(tc.tile_pool(name="wst", bufs=3))
            stgp = ctx.enter_context(tc.tile_pool(name="stgp", bufs=2))
            drp = ctx.enter_context(tc.tile_pool(name="drp", bufs=2, space="DRAM"))
            simp = ctx.enter_context(tc.tile_pool(name="simp", bufs=3, space="PSUM"))
            accp = ctx.enter_context(tc.tile_pool(name="accp", bufs=2, space="PSUM"))

            # ---- constant loads. One packed DMA for the small weights so
            # projections start as soon as xt0 lands; mask tiles split across
            # both HWDGE queues (the whole 8MB mask is consumed by the first
            # q-block, so its arrival races the mul pipeline); wo last. ----
            wp_sb = const.tile([128, 3 * 64 + 2], F32, tag="wp")
            nc.sync.dma_start(out=wp_sb.bitcast(F32R), in_=wp_d[:, :].bitcast(F32R))
            wq_sb = wp_sb[:, 0:64]
            wk_sb = wp_sb[:, 64:128]
            wv_sb = wp_sb[:, 128:192]
            bq_sb = wp_sb[:, 192:193]
            bk_sb = wp_sb[:, 193:194]
            xt_sb = [
                const.tile([128, S], F32, tag=f"xt{g}", name=f"xt{g}")
                for g in range(2)
            ]
            nc.sync.dma_start(
                out=xt_sb[0][:, 0:512].bitcast(F32R),
                in_=xt_d[0:128, 0:512].bitcast(F32R),
            )
            nc.sync.dma_start(
                out=xt_sb[0][:, 512:S].bitcast(F32R),
                in_=xt_d[0:128, 512:S].bitcast(F32R),
            )
            # mask is qb-major: the first 4 chunks cover the whole first
            # q-block, so QK's mask-muls start ~6us in instead of ~22us
            mask_sb = const.tile([128, NKC * S], BF16, tag="mask")
            for kc in range(4):
                eng = nc.scalar if kc % 2 == 0 else nc.sync
                eng.dma_start(
                    out=mask_sb[:, kc * S : (kc + 1) * S],
                    in_=mask_d[:, kc * S : (kc + 1) * S],
                )
            nc.scalar.dma_start(
                out=xt_sb[1].bitcast(F32R),
                in_=xt_d[128:256, :].bitcast(F32R),
            )
            for kc in range(4, NKC):
                eng = nc.scalar if kc % 2 == 0 else nc.sync
                eng.dma_start(
                    out=mask_sb[:, kc * S : (kc + 1) * S],
                    in_=mask_d[:, kc * S : (kc + 1) * S],
                )
            wo_sb = []
            for g in range(2):
                t = const.tile([128, E], BF16, tag=f"wo{g}")
                nc.scalar.dma_start(out=t, in_=wo_d[g * 128 : (g + 1) * 128, :])
                wo_sb.append(t)

            for rep in range(repeat):
                outT2 = []
                for g in range(2):
                    # ---- projections for group g (pairs at xt rows 0-63 / 64-127) ----
                    qt = [qkp.tile([64, S], BF16, tag=f"qt{p}", name=f"qt_g{g}_{p}_r{rep}") for p in range(2)]
                    kt = [qkp.tile([64, S], BF16, tag=f"kt{p}", name=f"kt_g{g}_{p}_r{rep}") for p in range(2)]
                    for sp in range(S // 1024):
                        ssl = slice(sp * 1024, (sp + 1) * 1024)
                        for p01 in range(2):
                            rsl = slice(p01 * 64, p01 * 64 + 64)
                            qps = simp.tile([64, 1024], F32, tag="sim")
                            for h in range(2):
                                hsl = slice(
                                    sp * 1024 + h * 512, sp * 1024 + (h + 1) * 512
                                )
                                nc.tensor.matmul(
                                    qps[:, h * 512 : (h + 1) * 512],
                                    wq_sb[rsl, :].bitcast(F32R),
                                    xt_sb[g][rsl, hsl].bitcast(F32R),
                                    start=True,
                                    stop=True,
                                )
                            nc.scalar.activation(
                                qt[p01][:, ssl], qps[:, :], Ident, bias=bq_sb[0:64, :]
                            )
                            kps = simp.tile([64, 1024], F32, tag="sim")
                            for h in range(2):
                                hsl = slice(
                                    sp * 1024 + h * 512, sp * 1024 + (h + 1) * 512
                                )
                                nc.tensor.matmul(
                                    kps[:, h * 512 : (h + 1) * 512],
                                    wk_sb[rsl, :].bitcast(F32R),
                                    xt_sb[g][rsl, hsl].bitcast(F32R),
                                    start=True,
                                    stop=True,
                                )
                            nc.scalar.activation(
                                kt[p01][:, ssl], kps[:, :], Ident, bias=bk_sb[0:64, :]
                            )

                    # ---- attention: qb outer, pair inner. V matmuls for each
                    # pair are emitted inside the first qb, after that pair's
                    # first QK/mask-mul batch, so the DVE pipeline starts
                    # before the PE spends ~7us on V (priority = emit order;
                    # V is only needed at the first PV, which comes later). ----
                    v_sb = [None, None]
                    ot2 = ot2p.tile([128, S], BF16, tag="outt2")
                    for qb in range(S // QB):
                        qsl = slice(qb * QB, (qb + 1) * QB)
                        for p01 in range(2):
                            pt = ptp.tile([128, NKC * QB], BF16, tag="pt")
                            for kc2 in range(NKC // 2):
                                sm = simp.tile([128, 2 * QB], F32, tag="sim")
                                for j in range(2):
                                    kc = 2 * kc2 + j
                                    nc.tensor.matmul(
                                        sm[:, j * QB : (j + 1) * QB],
                                        kt[p01][:, kc * 128 : (kc + 1) * 128],
                                        qt[p01][:, qsl],
                                        start=True,
                                        stop=True,
                                    )
                                moff = (qb * (NKC // 2) + kc2) * 2 * QB
                                nc.vector.tensor_mul(
                                    pt[:, 2 * kc2 * QB : (2 * kc2 + 2) * QB],
                                    sm[:, :],
                                    mask_sb[:, moff : moff + 2 * QB],
                                )
                            if qb == 0:
                                vt = vp.tile(
                                    [128, NKC * 65], BF16, tag="vsb",
                                    name=f"v_g{g}_{p01}_r{rep}",
                                )
                                ones_ap = vt.rearrange("p (c k) -> p c k", k=65)[
                                    :, :, 64:65
                                ]
                                nc.gpsimd.memset(ones_ap, 1.0)
                                rsl = slice(p01 * 64, p01 * 64 + 64)
                                for vc4 in range(NKC // 4):
                                    vps = accp.tile([128, 256], F32, tag="acc")
                                    for j in range(4):
                                        sc = vc4 * 4 + j
                                        nc.tensor.matmul(
                                            vps[:, j * 64 : (j + 1) * 64],
                                            xt_sb[g][
                                                rsl, sc * 128 : (sc + 1) * 128
                                            ].bitcast(F32R),
                                            wv_sb[rsl, :].bitcast(F32R),
                                            start=True,
                                            stop=True,
                                        )
                                    nc.vector.tensor_copy(
                                        vt.rearrange("p (c k) -> p c k", k=65)[
                                            :, vc4 * 4 : (vc4 + 1) * 4, 0:64
                                        ],
                                        vps.rearrange("p (c k) -> p c k", k=64),
                                    )
                                v_sb[p01] = vt
                            for e2 in range(NEXP):
                                half = slice(
                                    e2 * (NKC * QB // NEXP),
                                    (e2 + 1) * (NKC * QB // NEXP),
                                )
                                nc.scalar.activation(pt[:, half], pt[:, half], Exp)
                            pv = accp.tile([65, QB], F32, tag="acc")
                            for kc in range(NKC):
                                nc.tensor.matmul(
                                    pv[:, :],
                                    v_sb[p01][:, kc * 65 : (kc + 1) * 65],
                                    pt[:, kc * QB : (kc + 1) * QB],
                                    start=(kc == 0),
                                    stop=(kc == NKC - 1),
                                )
                            # denominator -> reciprocal -> broadcast
                            den = denp_pool.tile([1, QB], F32, tag="den")
                            nc.scalar.activation(den, pv[64:65, :], Copy)
                            dden = drp.tile([1, QB], F32, tag="dden")
                            nc.sync.dma_start(out=dden, in_=den)
                            dpk = denp_pool.tile([128, QB // 128], F32, tag="dpk")
                            nc.sync.dma_start(
                                out=dpk,
                                in_=dden.rearrange("a (p f) -> (a p) f", p=128),
                            )
                            rpk = denp_pool.tile([128, QB // 128], F32, tag="rpk")
                            nc.vector.reciprocal(rpk, dpk)
                            drec = drp.tile([1, QB], F32, tag="drec")
                            nc.sync.dma_start(
                                out=drec.rearrange("a (p f) -> (a p) f", p=128),
                                in_=rpk,
                            )
                            recb = denp_pool.tile([64, QB], F32, tag="recb")
                            nc.sync.dma_start(
                                out=recb,
                                in_=bass.AP(
                                    tensor=drec.tensor,
                                    offset=drec.offset,
                                    ap=[[0, 64]] + [list(a) for a in drec.ap[1:]],
                                ),
                            )
                            ot = otp.tile([64, QB], BF16, tag="outt")
                            nc.vector.tensor_mul(ot, pv[0:64, :], recb)
                            # pack into the group tile (partition-shift DMA)
                            nc.scalar.dma_start(
                                out=ot2[p01 * 64 : p01 * 64 + 64, qsl], in_=ot
                            )
                    outT2.append(ot2)

                # ---- output projection (partial, this core's 4 heads) ----
                for qc in range(S // 128):
                    wo_ps = simp.tile([128, E], F32, tag="sim")
                    for gi in range(2):
                        for e2 in range(2):
                            nc.tensor.matmul(
                                wo_ps[:, e2 * 512 : (e2 + 1) * 512],
                                outT2[gi][:, qc * 128 : (qc + 1) * 128],
                                wo_sb[gi][:, e2 * 512 : (e2 + 1) * 512],
                                start=(gi == 0),
                                stop=(gi == 1),
                            )
                    ost = wst.tile([128, E], BF16, tag="wst")
                    if qc % 2 == 0:
                        nc.scalar.activation(ost, wo_ps[:, :], Copy)
                    else:
                        nc.vector.tensor_copy(ost, wo_ps[:, :])
                    eng = nc.sync if qc % 2 == 0 else nc.scalar
                    eng.dma_start(out=out_d[qc * 128 : (qc + 1) * 128, :], in_=ost)
    nc.finalize()
    return nc


def _build_runner(repeat=1):
    """Compile once. Returns an object with:
    - prep(in_maps): host arrays -> device-resident committed args
    - make_zeros(): device-side zero output buffers (donated per exec)
    - exec_device(args): one bass execution -> sharded partials (blocked)
    - reduce_device(partials): on-device cross-core sum -> (B*S, E)
    - run(in_maps): full host->host pipeline (correctness path)
    """
    import jax
    import jax.numpy as jnp
    import numpy as _np
    from jax.experimental.shard_map import shard_map
    from jax.sharding import Mesh, NamedSharding, PartitionSpec

    from concourse import mybir
    from concourse.bass2jax import (
        _bass_exec_p,
        install_neuronx_cc_hook,
        partition_id_tensor,
    )

    nc = _build_nc(repeat=repeat)
    install_neuronx_cc_hook()
    partition_name = nc.partition_id_tensor.name if nc.partition_id_tensor else None

    replicated = {"maskt", "wpack"}

    in_names, out_names, out_avals, out_shapes, out_dtypes = [], [], [], [], []
    for alloc in nc.m.functions[0].allocations:
        if not isinstance(alloc, mybir.MemoryLocationSet):
            continue
        name = alloc.memorylocations[0].name
        if alloc.kind == "ExternalInput":
            if name != partition_name:
                in_names.append(name)
        elif alloc.kind == "ExternalOutput":
            out_names.append(name)
            shape = tuple(alloc.tensor_shape)
            dtype = mybir.dt.np(alloc.dtype)
            out_avals.append(jax.core.ShapedArray(shape, dtype))
            out_shapes.append(shape)
            out_dtypes.append(dtype)

    n_params = len(in_names)
    n_outs = len(out_names)
    all_in_names = list(in_names) + list(out_names)
    if partition_name is not None:
        all_in_names.append(partition_name)
    donate = tuple(range(n_params, n_params + n_outs))

    def _body(*args):
        operands = list(args)
        if partition_name is not None:
            operands.append(partition_id_tensor())
        outs = _bass_exec_p.bind(
            *operands,
            out_avals=tuple(out_avals),
            in_names=tuple(all_in_names),
            out_names=tuple(out_names),
            lowering_input_output_aliases=(),
            sim_require_finite=True,
            sim_require_nnan=True,
            nc=nc,
        )
        return tuple(outs)

    devices = jax.devices()[:N_CORES]
    mesh = Mesh(_np.asarray(devices), ("core",))
    shard0 = NamedSharding(mesh, PartitionSpec("core"))
    srepl = NamedSharding(mesh, PartitionSpec())
    in_specs = tuple(
        PartitionSpec() if name in replicated else PartitionSpec("core")
        for name in in_names
    ) + (PartitionSpec("core"),) * n_outs
    out_specs = (PartitionSpec("core"),) * n_outs

    sharded = jax.jit(
        shard_map(
            _body, mesh=mesh, in_specs=in_specs, out_specs=out_specs,
            check_rep=False,
        ),
        donate_argnums=donate,
        keep_unused=True,
    )

    _zeros = jax.jit(
        lambda: tuple(
            jnp.zeros((N_CORES * s[0], *s[1:]), d)
            for s, d in zip(out_shapes, out_dtypes)
        ),
        out_shardings=(shard0,) * n_outs,
    )

    _reduce = jax.jit(
        lambda p: p.reshape(B, 4, S, E).sum(axis=1).reshape(B * S, E),
        out_shardings=shard0,
    )

    def prep(in_maps):
        args = []
        for name in in_names:
            if name in replicated:
                arr = _np.asarray(in_maps[0][name])
                args.append(jax.device_put(arr, srepl))
            else:
                arr = _np.concatenate(
                    [_np.asarray(m[name]) for m in in_maps], axis=0
                )
                args.append(jax.device_put(arr, shard0))
        return args

    def make_zeros():
        return _zeros()

    def exec_device(args, zeros=None):
        if zeros is None:
            zeros = _zeros()
        outs = sharded(*args, *zeros)
        return jax.block_until_ready(outs[0])

    def exec_async(args, zeros):
        return sharded(*args, *zeros)[0]

    def reduce_device(partials):
        return jax.block_until_ready(_reduce(partials))

    def run(in_maps):
        partials = exec_device(prep(in_maps))
        return _np.asarray(reduce_device(partials))  # (B*S, E)

    class R:
        pass

    r = R()
    r.prep = prep
    r.make_zeros = make_zeros
    r.exec_device = exec_device
    r.exec_async = exec_async
    r.reduce_device = reduce_device
    r.run = run
    return r


def _runtime(repeat=1):
    if repeat not in _RUNTIME:
        _RUNTIME[repeat] = _build_runner(repeat=repeat)
    return _RUNTIME[repeat]


def make_in_maps(x, mask, Wq, bq, Wk, bk, Wv, bv, Wo, bo):
    bf16 = ml_dtypes.bfloat16
    x = np.asarray(x, np.float32)
    m = np.asarray(mask, np.float32).T  # [k, q]
    # device consumes [128, (kc2, qb, j, ql)] blocks so each DVE
    # mask-multiply reads one flat [128, 2*QB] span
    maskT = np.ascontiguousarray(
        m.reshape(NKC // 2, 2, 128, S // QB, QB)
        .transpose(2, 3, 0, 1, 4)
        .reshape(128, NKC * S)
    ).astype(bf16)
    wq_s = (np.asarray(Wq, np.float32) * SCALE).astype(np.float32)
    bq_s = (np.asarray(bq, np.float32) * SCALE).astype(np.float32)
    wq2 = np.concatenate([wq_s, wq_s], axis=0)
    wk2 = np.concatenate([np.asarray(Wk, np.float32)] * 2, axis=0)
    wv2 = np.concatenate([np.asarray(Wv, np.float32)] * 2, axis=0)
    bq2 = np.concatenate([bq_s, bq_s])[:, None].astype(np.float32)
    bk2 = np.concatenate([np.asarray(bk, np.float32)] * 2)[:, None].astype(np.float32)
    wpack = np.ascontiguousarray(
        np.concatenate([wq2, wk2, wv2, bq2, bk2], axis=1), np.float32
    )

    in_maps = []
    for c in range(N_CORES):
        b = c // 4
        h0 = (c % 4) * HPC
        r0 = h0 * HD
        xt = np.ascontiguousarray(x[b].T[r0 : r0 + HPC * HD, :])
        wo = np.ascontiguousarray(np.asarray(Wo, np.float32)[r0 : r0 + HPC * VD, :]).astype(bf16)
        in_maps.append(
            {
                "xt": xt,
                "maskt": maskT,
                "wpack": wpack,
                "wo": wo,
            }
        )
    return in_maps


def kernel(x, mask, Wq, bq, Wk, bk, Wv, bv, Wo, bo):
    r = _runtime()
    in_maps = make_in_maps(x, mask, Wq, bq, Wk, bk, Wv, bv, Wo, bo)
    flat = r.run(in_maps)  # (B*S, E), per-batch partials already summed
    Wo32 = np.asarray(Wo, np.float32)
    crow = np.asarray(bo, np.float32) + np.tile(np.asarray(bv, np.float32), H) @ Wo32
    out = flat.reshape(B, S, E) + crow[None, None, :]
    return out.astype(np.float32)



# revision 31
# speedup vs baseline: 1.0471x; 1.0471x over previous
"""Multi-head attention (B=2, S=2048, E=1024, H=16) on 8 TRN2 NeuronCores.

Sharding: batch x head-group. Core c handles batch c//4 and heads
(c%4)*4 .. +3. Each core computes Q/K/V projections, masked softmax
attention, and a partial output projection against its 4-head slice of
Wo (rows h0*64 .. h0*64+255). Host sums the 4 partials per batch and
adds the constant row bo + tile(bv,H) @ Wo (the V-bias contribution
commutes through softmax normalization: PV(v0 + 1*bv) =
PV(v0) + denom*bv, so after normalization it is exactly +bv per head,
which flows through Wo as a constant row).

Device layout is fully transposed (feature-major): the host supplies
x^T and a pre-arranged mask^T so no on-device transposes are needed.
  - qT/kT [KD, S] per head (bf16), computed as W.T @ x^T (f32r matmul;
    both pairs of a group read xt at base partitions 0/64, with the
    weight rows duplicated [W;W] so lhsT/rhs base partitions match)
  - simT [k, q] tiles in PSUM; mask-multiplied on DVE (psum x bf16 ->
    bf16); exp on ACT in chunks
  - PV with stationary [s_chunk, 65] = [V | ones] bf16: row 64 of the
    psum accumulator is the softmax denominator for free
  - reciprocal via a [1,N] -> [128,N/128] DRAM-bounce repack (DVE
    reciprocal is per-lane; a row on 1 partition would be 8 cyc/elem)
  - rec broadcast to 64 partitions via a partition-step-0 DRAM read
  - per-pair outT rows packed into a [128, S] group tile via
    partition-shift SBUF->SBUF DMA; Wo contraction then runs K=128
    (two heads per matmul) with bf16 operands; partials exported bf16
"""
import sys

if "/opt/trn_rl_repo" not in sys.path:
    sys.path.insert(0, "/opt/trn_rl_repo")

from contextlib import ExitStack

import ml_dtypes
import numpy as np

B, S, E = 2, 2048, 1024
H = 16
HD = 64
KD = 64
VD = 64
SCALE = 1.0 / np.float32(np.sqrt(np.float32(KD)))
N_CORES = 8
HPC = H // 4  # heads per core = 4
QB = 512  # q-block width
NKC = S // 128  # 16 k-chunks
NEXP = 4  # exp ops per (pair, qb) pt tile

_RUNTIME = {}


def _build_nc(repeat=1):
    import concourse.bass as bass
    import concourse.tile as tile
    from concourse import mybir, bacc

    F32 = mybir.dt.float32
    F32R = mybir.dt.float32r
    BF16 = mybir.dt.bfloat16
    Copy = mybir.ActivationFunctionType.Copy
    Ident = mybir.ActivationFunctionType.Identity
    Exp = mybir.ActivationFunctionType.Exp

    nc = bacc.Bacc("TRN2")
    xt_d = nc.dram_tensor("xt", (4 * HD, S), F32, kind="ExternalInput")
    mask_d = nc.dram_tensor("maskt", (128, NKC * S), BF16, kind="ExternalInput")
    wp_d = nc.dram_tensor("wpack", (128, 3 * 64 + 2), F32, kind="ExternalInput")
    wo_d = nc.dram_tensor("wo", (4 * VD, E), BF16, kind="ExternalInput")
    out_d = nc.dram_tensor("partial", (S, E), BF16, kind="ExternalOutput")

    with tile.TileContext(nc) as tc:
        with ExitStack() as ctx:
            const = ctx.enter_context(tc.tile_pool(name="const", bufs=1))
            qkp = ctx.enter_context(tc.tile_pool(name="qkp", bufs=2))
            vp = ctx.enter_context(tc.tile_pool(name="vp", bufs=3))
            ptp = ctx.enter_context(tc.tile_pool(name="ptp", bufs=2))
            otp = ctx.enter_context(tc.tile_pool(name="otp", bufs=4))
            ot2p = ctx.enter_context(tc.tile_pool(name="ot2p", bufs=2))
            denp_pool = ctx.enter_context(tc.tile_pool(name="denp", bufs=3))
            wst = ctx.enter_context(tc.tile_pool(name="wst", bufs=3))
            stgp = ctx.enter_context(tc.tile_pool(name="stgp", bufs=2))
            drp = ctx.enter_context(tc.tile_pool(name="drp", bufs=2, space="DRAM"))
            simp = ctx.enter_context(tc.tile_pool(name="simp", bufs=3, space="PSUM"))
            accp = ctx.enter_context(tc.tile_pool(name="accp", bufs=2, space="PSUM"))

            # ---- constant loads. One packed DMA for the small weights so
            # projections start as soon as xt0 lands; mask tiles split across
            # both HWDGE queues (the whole 8MB mask is consumed by the first
            # q-block, so its arrival races the mul pipeline); wo last. ----
            wp_sb = const.tile([128, 3 * 64 + 2], F32, tag="wp")
            nc.sync.dma_start(out=wp_sb.bitcast(F32R), in_=wp_d[:, :].bitcast(F32R))
            # HAM warm-up: dummy matmuls fill the dead input-DMA window so
            # the first real projections start at full PE clock. No consumers;
            # the first proj tile reuses the slot after the last write.
            warm = simp.tile([128, 128], F32, tag="sim", name="warmup")
            for _ in range(24):
                nc.tensor.matmul(
                    warm[0:64, :],
                    wp_sb[0:64, 0:64].bitcast(F32R),
                    wp_sb[0:64, 64:192].bitcast(F32R),
                    start=True, stop=True,
                )
            wq_sb = wp_sb[:, 0:64]
            wk_sb = wp_sb[:, 64:128]
            wv_sb = wp_sb[:, 128:192]
            bq_sb = wp_sb[:, 192:193]
            bk_sb = wp_sb[:, 193:194]
            xt_sb = []
            for g in range(2):
                t = const.tile([128, S], F32, tag=f"xt{g}")
                nc.sync.dma_start(
                    out=t.bitcast(F32R),
                    in_=xt_d[g * 128 : (g + 1) * 128, :].bitcast(F32R),
                )
                xt_sb.append(t)
            mask_sb = const.tile([128, NKC * S], BF16, tag="mask")
            for kc in range(NKC):
                eng = nc.scalar if kc % 2 == 0 else nc.sync
                eng.dma_start(
                    out=mask_sb[:, kc * S : (kc + 1) * S],
                    in_=mask_d[:, kc * S : (kc + 1) * S],
                )
            wo_sb = []
            for g in range(2):
                t = const.tile([128, E], BF16, tag=f"wo{g}")
                nc.scalar.dma_start(out=t, in_=wo_d[g * 128 : (g + 1) * 128, :])
                wo_sb.append(t)

            for rep in range(repeat):
                outT2 = []
                for g in range(2):
                    # ---- projections for group g (pairs at xt rows 0-63 / 64-127) ----
                    qt = [qkp.tile([64, S], BF16, tag=f"qt{p}", name=f"qt_g{g}_{p}_r{rep}") for p in range(2)]
                    kt = [qkp.tile([64, S], BF16, tag=f"kt{p}", name=f"kt_g{g}_{p}_r{rep}") for p in range(2)]
                    for sp in range(S // 1024):
                        ssl = slice(sp * 1024, (sp + 1) * 1024)
                        for p01 in range(2):
                            rsl = slice(p01 * 64, p01 * 64 + 64)
                            qps = simp.tile([64, 1024], F32, tag="sim")
                            for h in range(2):
                                hsl = slice(
                                    sp * 1024 + h * 512, sp * 1024 + (h + 1) * 512
                                )
                                nc.tensor.matmul(
                                    qps[:, h * 512 : (h + 1) * 512],
                                    wq_sb[rsl, :].bitcast(F32R),
                                    xt_sb[g][rsl, hsl].bitcast(F32R),
                                    start=True,
                                    stop=True,
                                )
                            nc.scalar.activation(
                                qt[p01][:, ssl], qps[:, :], Ident, bias=bq_sb[0:64, :]
                            )
                            kps = simp.tile([64, 1024], F32, tag="sim")
                            for h in range(2):
                                hsl = slice(
                                    sp * 1024 + h * 512, sp * 1024 + (h + 1) * 512
                                )
                                nc.tensor.matmul(
                                    kps[:, h * 512 : (h + 1) * 512],
                                    wk_sb[rsl, :].bitcast(F32R),
                                    xt_sb[g][rsl, hsl].bitcast(F32R),
                                    start=True,
                                    stop=True,
                                )
                            nc.scalar.activation(
                                kt[p01][:, ssl], kps[:, :], Ident, bias=bk_sb[0:64, :]
                            )

                    # ---- attention: qb outer, pair inner. V matmuls for each
                    # pair are emitted inside the first qb, after that pair's
                    # first QK/mask-mul batch, so the DVE pipeline starts
                    # before the PE spends ~7us on V (priority = emit order;
                    # V is only needed at the first PV, which comes later). ----
                    v_sb = [None, None]
                    ot2 = ot2p.tile([128, S], BF16, tag="outt2")
                    for qb in range(S // QB):
                        qsl = slice(qb * QB, (qb + 1) * QB)
                        for p01 in range(2):
                            pt = ptp.tile([128, NKC * QB], BF16, tag="pt")
                            for kc2 in range(NKC // 2):
                                sm = simp.tile([128, 2 * QB], F32, tag="sim")
                                for j in range(2):
                                    kc = 2 * kc2 + j
                                    nc.tensor.matmul(
                                        sm[:, j * QB : (j + 1) * QB],
                                        kt[p01][:, kc * 128 : (kc + 1) * 128],
                                        qt[p01][:, qsl],
                                        start=True,
                                        stop=True,
                                    )
                                moff = (kc2 * (S // QB) + qb) * 2 * QB
                                if kc2 == 5 and (qb + p01) % 2 == 0:
                                    # rebalance: DVE is the busiest engine, so
                                    # route a few mask-muls via ACT (psum->sbuf
                                    # bf16) + the otherwise-idle GpSimd
                                    stg = stgp.tile([128, 2 * QB], BF16, tag="stg")
                                    nc.scalar.activation(stg, sm[:, :], Copy)
                                    nc.gpsimd.tensor_mul(
                                        pt[:, 2 * kc2 * QB : (2 * kc2 + 2) * QB],
                                        stg,
                                        mask_sb[:, moff : moff + 2 * QB],
                                    )
                                else:
                                    nc.vector.tensor_mul(
                                        pt[:, 2 * kc2 * QB : (2 * kc2 + 2) * QB],
                                        sm[:, :],
                                        mask_sb[:, moff : moff + 2 * QB],
                                    )
                            if qb == 0:
                                vt = vp.tile(
                                    [128, NKC * 65], BF16, tag="vsb",
                                    name=f"v_g{g}_{p01}_r{rep}",
                                )
                                ones_ap = vt.rearrange("p (c k) -> p c k", k=65)[
                                    :, :, 64:65
                                ]
                                nc.gpsimd.memset(ones_ap, 1.0)
                                rsl = slice(p01 * 64, p01 * 64 + 64)
                                for vc4 in range(NKC // 4):
                                    vps = accp.tile([128, 256], F32, tag="acc")
                                    for j in range(4):
                                        sc = vc4 * 4 + j
                                        nc.tensor.matmul(
                                            vps[:, j * 64 : (j + 1) * 64],
                                            xt_sb[g][
                                                rsl, sc * 128 : (sc + 1) * 128
                                            ].bitcast(F32R),
                                            wv_sb[rsl, :].bitcast(F32R),
                                            start=True,
                                            stop=True,
                                        )
                                    nc.vector.tensor_copy(
                                        vt.rearrange("p (c k) -> p c k", k=65)[
                                            :, vc4 * 4 : (vc4 + 1) * 4, 0:64
                                        ],
                                        vps.rearrange("p (c k) -> p c k", k=64),
                                    )
                                v_sb[p01] = vt
                            for e2 in range(NEXP):
                                half = slice(
                                    e2 * (NKC * QB // NEXP),
                                    (e2 + 1) * (NKC * QB // NEXP),
                                )
                                nc.scalar.activation(pt[:, half], pt[:, half], Exp)
                            pv = accp.tile([65, QB], F32, tag="acc")
                            for kc in range(NKC):
                                nc.tensor.matmul(
                                    pv[:, :],
                                    v_sb[p01][:, kc * 65 : (kc + 1) * 65],
                                    pt[:, kc * QB : (kc + 1) * QB],
                                    start=(kc == 0),
                                    stop=(kc == NKC - 1),
                                )
                            # denominator -> reciprocal -> broadcast
                            den = denp_pool.tile([1, QB], F32, tag="den")
                            nc.scalar.activation(den, pv[64:65, :], Copy)
                            dden = drp.tile([1, QB], F32, tag="dden")
                            nc.sync.dma_start(out=dden, in_=den)
                            dpk = denp_pool.tile([128, QB // 128], F32, tag="dpk")
                            nc.sync.dma_start(
                                out=dpk,
                                in_=dden.rearrange("a (p f) -> (a p) f", p=128),
                            )
                            rpk = denp_pool.tile([128, QB // 128], F32, tag="rpk")
                            nc.vector.reciprocal(rpk, dpk)
                            drec = drp.tile([1, QB], F32, tag="drec")
                            nc.sync.dma_start(
                                out=drec.rearrange("a (p f) -> (a p) f", p=128),
                                in_=rpk,
                            )
                            recb = denp_pool.tile([64, QB], F32, tag="recb")
                            nc.sync.dma_start(
                                out=recb,
                                in_=bass.AP(
                                    tensor=drec.tensor,
                                    offset=drec.offset,
                                    ap=[[0, 64]] + [list(a) for a in drec.ap[1:]],
                                ),
                            )
                            ot = otp.tile([64, QB], BF16, tag="outt")
                            nc.vector.tensor_mul(ot, pv[0:64, :], recb)
                            # pack into the group tile (partition-shift DMA)
                            nc.scalar.dma_start(
                                out=ot2[p01 * 64 : p01 * 64 + 64, qsl], in_=ot
                            )
                    outT2.append(ot2)

                # ---- output projection (partial, this core's 4 heads) ----
                for qc in range(S // 128):
                    wo_ps = simp.tile([128, E], F32, tag="sim")
                    for gi in range(2):
                        for e2 in range(2):
                            nc.tensor.matmul(
                                wo_ps[:, e2 * 512 : (e2 + 1) * 512],
                                outT2[gi][:, qc * 128 : (qc + 1) * 128],
                                wo_sb[gi][:, e2 * 512 : (e2 + 1) * 512],
                                start=(gi == 0),
                                stop=(gi == 1),
                            )
                    ost = wst.tile([128, E], BF16, tag="wst")
                    if qc % 2 == 0:
                        nc.scalar.activation(ost, wo_ps[:, :], Copy)
                    else:
                        nc.vector.tensor_copy(ost, wo_ps[:, :])
                    eng = nc.sync if qc % 2 == 0 else nc.scalar
                    eng.dma_start(out=out_d[qc * 128 : (qc + 1) * 128, :], in_=ost)
    nc.finalize()
    return nc


def _build_runner(repeat=1):
    """Compile once. Returns an object with:
    - prep(in_maps): host arrays -> device-resident committed args
    - make_zeros(): device-side zero output buffers (donated per exec)
    - exec_device(args): one bass execution -> sharded partials (blocked)
    - reduce_device(partials): on-device cross-core sum -> (B*S, E)
    - run(in_maps): full host->host pipeline (correctness path)
    """
    import jax
    import jax.numpy as jnp
    import numpy as _np
    from jax.experimental.shard_map import shard_map
    from jax.sharding import Mesh, NamedSharding, PartitionSpec

    from concourse import mybir
    from concourse.bass2jax import (
        _bass_exec_p,
        install_neuronx_cc_hook,
        partition_id_tensor,
    )

    nc = _build_nc(repeat=repeat)
    install_neuronx_cc_hook()
    partition_name = nc.partition_id_tensor.name if nc.partition_id_tensor else None

    replicated = {"maskt", "wpack"}

    in_names, out_names, out_avals, out_shapes, out_dtypes = [], [], [], [], []
    for alloc in nc.m.functions[0].allocations:
        if not isinstance(alloc, mybir.MemoryLocationSet):
            continue
        name = alloc.memorylocations[0].name
        if alloc.kind == "ExternalInput":
            if name != partition_name:
                in_names.append(name)
        elif alloc.kind == "ExternalOutput":
            out_names.append(name)
            shape = tuple(alloc.tensor_shape)
            dtype = mybir.dt.np(alloc.dtype)
            out_avals.append(jax.core.ShapedArray(shape, dtype))
            out_shapes.append(shape)
            out_dtypes.append(dtype)

    n_params = len(in_names)
    n_outs = len(out_names)
    all_in_names = list(in_names) + list(out_names)
    if partition_name is not None:
        all_in_names.append(partition_name)
    donate = tuple(range(n_params, n_params + n_outs))

    def _body(*args):
        operands = list(args)
        if partition_name is not None:
            operands.append(partition_id_tensor())
        outs = _bass_exec_p.bind(
            *operands,
            out_avals=tuple(out_avals),
            in_names=tuple(all_in_names),
            out_names=tuple(out_names),
            lowering_input_output_aliases=(),
            sim_require_finite=True,
            sim_require_nnan=True,
            nc=nc,
        )
        return tuple(outs)

    devices = jax.devices()[:N_CORES]
    mesh = Mesh(_np.asarray(devices), ("core",))
    shard0 = NamedSharding(mesh, PartitionSpec("core"))
    srepl = NamedSharding(mesh, PartitionSpec())
    in_specs = tuple(
        PartitionSpec() if name in replicated else PartitionSpec("core")
        for name in in_names
    ) + (PartitionSpec("core"),) * n_outs
    out_specs = (PartitionSpec("core"),) * n_outs

    sharded = jax.jit(
        shard_map(
            _body, mesh=mesh, in_specs=in_specs, out_specs=out_specs,
            check_rep=False,
        ),
        donate_argnums=donate,
        keep_unused=True,
    )

    _zeros = jax.jit(
        lambda: tuple(
            jnp.zeros((N_CORES * s[0], *s[1:]), d)
            for s, d in zip(out_shapes, out_dtypes)
        ),
        out_shardings=(shard0,) * n_outs,
    )

    _reduce = jax.jit(
        lambda p: p.reshape(B, 4, S, E).sum(axis=1).reshape(B * S, E),
        out_shardings=shard0,
    )

    def prep(in_maps):
        args = []
        for name in in_names:
            if name in replicated:
                arr = _np.asarray(in_maps[0][name])
                args.append(jax.device_put(arr, srepl))
            else:
                arr = _np.concatenate(
                    [_np.asarray(m[name]) for m in in_maps], axis=0
                )
                args.append(jax.device_put(arr, shard0))
        return args

    def make_zeros():
        return _zeros()

    def exec_device(args, zeros=None):
        if zeros is None:
            zeros = _zeros()
        outs = sharded(*args, *zeros)
        return jax.block_until_ready(outs[0])

    def exec_async(args, zeros):
        return sharded(*args, *zeros)[0]

    def reduce_device(partials):
        return jax.block_until_ready(_reduce(partials))

    def run(in_maps):
        partials = exec_device(prep(in_maps))
        return _np.asarray(reduce_device(partials))  # (B*S, E)

    class R:
        pass

    r = R()
    r.prep = prep
    r.make_zeros = make_zeros
    r.exec_device = exec_device
    r.exec_async = exec_async
    r.reduce_device = reduce_device
    r.run = run
    return r


def _runtime(repeat=1):
    if repeat not in _RUNTIME:
        _RUNTIME[repeat] = _build_runner(repeat=repeat)
    return _RUNTIME[repeat]


def make_in_maps(x, mask, Wq, bq, Wk, bk, Wv, bv, Wo, bo):
    bf16 = ml_dtypes.bfloat16
    x = np.asarray(x, np.float32)
    m = np.asarray(mask, np.float32).T  # [k, q]
    # device consumes [128, (kc2, qb, j, ql)] blocks so each DVE
    # mask-multiply reads one flat [128, 2*QB] span
    maskT = np.ascontiguousarray(
        m.reshape(NKC // 2, 2, 128, S // QB, QB)
        .transpose(2, 0, 3, 1, 4)
        .reshape(128, NKC * S)
    ).astype(bf16)
    wq_s = (np.asarray(Wq, np.float32) * SCALE).astype(np.float32)
    bq_s = (np.asarray(bq, np.float32) * SCALE).astype(np.float32)
    wq2 = np.concatenate([wq_s, wq_s], axis=0)
    wk2 = np.concatenate([np.asarray(Wk, np.float32)] * 2, axis=0)
    wv2 = np.concatenate([np.asarray(Wv, np.float32)] * 2, axis=0)
    bq2 = np.concatenate([bq_s, bq_s])[:, None].astype(np.float32)
    bk2 = np.concatenate([np.asarray(bk, np.float32)] * 2)[:, None].astype(np.float32)
    wpack = np.ascontiguousarray(
        np.concatenate([wq2, wk2, wv2, bq2, bk2], axis=1), np.float32
    )

    in_maps = []
    for c in range(N_CORES):
        b = c // 4
        h0 = (c % 4) * HPC
        r0 = h0 * HD
        xt = np.ascontiguousarray(x[b].T[r0 : r0 + HPC * HD, :])
        wo = np.ascontiguousarray(np.asarray(Wo, np.float32)[r0 : r0 + HPC * VD, :]).astype(bf16)
        in_maps.append(
            {
                "xt": xt,
                "maskt": maskT,
                "wpack": wpack,
                "wo": wo,
            }
        )
    return in_maps


def kernel(x, mask, Wq, bq, Wk, bk, Wv, bv, Wo, bo):
    r = _runtime()
    in_maps = make_in_maps(x, mask, Wq, bq, Wk, bk, Wv, bv, Wo, bo)
    flat = r.run(in_maps)  # (B*S, E), per-batch partials already summed
    Wo32 = np.asarray(Wo, np.float32)
    crow = np.asarray(bo, np.float32) + np.tile(np.asarray(bv, np.float32), H) @ Wo32
    out = flat.reshape(B, S, E) + crow[None, None, :]
    return out.astype(np.float32)

